# revision 9
# baseline (speedup 1.0000x reference)
"""CapsuleLayer (dynamic routing) Trainium2 kernel — v3.

Full inputs -> batch-sharded over 8 NeuronCores -> full output.

Math (per sample b):
    ihat[i,c,o] = sum_d x[i,d] * W[i,c,d,o]
    bias = 0
    for r in 0..2:
        coup = softmax(bias, axis=c)
        s[c,o] = sum_i coup[i,c] * ihat[i,c,o]
        v = squash(s)
        if r < 2: bias[i,c] += sum_o ihat[i,c,o] * v[c,o]
    return v

Device layout (per core, 32 local samples, batch-tiles of 8):
    SBUF partition dim p = (b, i_sub): p = b*16 + i_sub, free dim (ig, c, o)
    with ig = i // 16 (72 groups).  ihat tile: [128, 72*10*16] bf16.

v3 perf notes:
  - Exp is the ONLY table-based scalar activation (one ACT_TABLE_LOAD
    total).  rsqrt in squash = bitcast magic-seed + Newton on DVE;
    1/z and 1/(1+n2) via DVE reciprocal.
  - o-reduction of ihat*v: bf16 2x-mode pairwise tree adds.
  - softmax state multiplicative: e *= exp(delta), no f32 bias tensor.
  - zsc coupling lhsT layout (g, b, c), all-bf16 step-1 -> 2x mask mult;
    rz materialized dense by scalar engine so coup mult also runs 2x.
  - einsum PSUM evacuation: 3 ig per 2KB PSUM bank, one scalar copy each.
  - batch tiles software-pipelined: emission order interleaves bt's so
    the DVE stream never waits on PE s-matmuls / scalar exp of the same
    chain; routing state lives in bufs=2 pools, ihat in bufs=3.
"""

import sys

if "/opt/trn_rl_repo" not in sys.path:
    sys.path.insert(0, "/opt/trn_rl_repo")

import numpy as np

B, I, D, C, O = 256, 1152, 8, 10, 16
NCORES = 8
BL = B // NCORES            # 32 local samples per core
NBT, BT = 4, 8              # batch tiles
ISUB = 16                   # i's per group
IG = I // ISUB              # 72 groups
CO = C * O                  # 160
NR = 3
EPS = 1e-7
XZ_CHUNK = 18               # ig's per xz DMA chunk
NCH = 4                     # bias-update chunks
F32 = np.float32
MAGIC = float(0x5F3759DF)   # rsqrt seed magic

USE_BF16 = True

_compiled = {}


def _build_program():
    import concourse.bacc as bacc
    import concourse.tile as tile
    import concourse.mybir as mybir
    import concourse.bass as bass

    f32 = mybir.dt.float32
    i32 = mybir.dt.int32
    lo = mybir.dt.bfloat16 if USE_BF16 else f32
    nc = bacc.Bacc("TRN2", target_bir_lowering=False, debug=False,
                   num_devices=NCORES)

    xz_t = nc.dram_tensor("xz", [NBT * IG, 128, 128], lo, kind="ExternalInput")
    xt_t = nc.dram_tensor("xt", [128, IG, BL], lo, kind="ExternalInput")
    w_t = nc.dram_tensor("w", [128, IG * CO], lo, kind="ExternalInput")
    cmask_t = nc.dram_tensor("cmask", [BT * C, CO], f32, kind="ExternalInput")
    maskz_t = nc.dram_tensor("maskz", [128, BT * C], lo, kind="ExternalInput")
    sel_t = nc.dram_tensor("sel", [BT * C, BT], f32, kind="ExternalInput")
    out_t = nc.dram_tensor("out", [BL, CO], f32, kind="ExternalOutput")
    vscr_t = nc.dram_tensor("vscr", [BL, CO], f32)   # internal scratch
    xz_ap, xt_ap, w_ap = xz_t.ap(), xt_t.ap(), w_t.ap()
    out_ap, vscr_ap = out_t.ap(), vscr_t.ap()

    AF = mybir.ActivationFunctionType
    ALU = mybir.AluOpType
    AX = mybir.AxisListType

    GN = IG // NCH           # 18 groups per chunk
    GC = GN * C              # 180

    with tile.TileContext(nc) as tc:
        from contextlib import ExitStack

        with ExitStack() as ctx:
            singles = ctx.enter_context(tc.tile_pool(name="singles", bufs=1))
            xzp = ctx.enter_context(tc.tile_pool(name="xzp", bufs=3))
            psum = ctx.enter_context(tc.tile_pool(name="psum", bufs=4, space="PSUM"))
            psm = ctx.enter_context(tc.tile_pool(name="psm", bufs=2, space="PSUM"))
            ihp = ctx.enter_context(tc.tile_pool(name="ihp", bufs=2))
            tch = ctx.enter_context(tc.tile_pool(name="tch", bufs=1))
            trp = ctx.enter_context(tc.tile_pool(name="trp", bufs=1))
            dp = ctx.enter_context(tc.tile_pool(name="dp", bufs=2))
            ep = ctx.enter_context(tc.tile_pool(name="ep", bufs=2))
            cp = ctx.enter_context(tc.tile_pool(name="cp", bufs=2))
            zp = ctx.enter_context(tc.tile_pool(name="zp", bufs=2))
            vp = ctx.enter_context(tc.tile_pool(name="vp", bufs=2))
            sm = ctx.enter_context(tc.tile_pool(name="sm", bufs=2))

            w_sb = singles.tile([128, IG * CO], lo)
            nc.sync.dma_start(out=w_sb, in_=w_ap)
            xt_sb = singles.tile([128, IG * BL], lo)
            nc.sync.dma_start(out=xt_sb,
                              in_=xt_ap.rearrange("p g b -> p (g b)"))
            cmask = singles.tile([BT * C, CO], f32)
            nc.sync.dma_start(out=cmask, in_=cmask_t.ap())
            maskz = singles.tile([128, BT * C], lo)
            nc.sync.dma_start(out=maskz, in_=maskz_t.ap())
            sel_sb = singles.tile([BT * C, BT], f32)
            nc.sync.dma_start(out=sel_sb, in_=sel_t.ap())

            def rsqrt_dve(pool, a, p, w, iters):
                """y ~= 1/sqrt(a) on DVE only (magic seed + Newton)."""
                sh = pool.tile([p, w], i32, name="rs_sh", tag="rs_sh")
                nc.vector.tensor_scalar(sh, a.bitcast(i32), 1, None,
                                        op0=ALU.logical_shift_right)
                yi = pool.tile([p, w], i32, name="rs_yi", tag="rs_yi")
                nc.vector.tensor_scalar(yi, sh, -1.0, MAGIC,
                                        op0=ALU.mult, op1=ALU.add)
                y = yi.bitcast(f32)
                for _ in range(iters):
                    t = pool.tile([p, w], f32, name="rs_t", tag="rs_t")
                    nc.vector.tensor_tensor(t, y, y, op=ALU.mult)
                    nc.vector.tensor_tensor(t, t, a, op=ALU.mult)
                    nc.vector.tensor_scalar(t, t, -0.5, 1.5,
                                            op0=ALU.mult, op1=ALU.add)
                    yn = pool.tile([p, w], f32, name="rs_yn", tag="rs_yn")
                    nc.vector.tensor_tensor(yn, y, t, op=ALU.mult)
                    y = yn
                return y

            def squash_scale(pool, n2, p, w, iters):
                """f = n2 / ((1+n2)*sqrt(n2+eps)), DVE only, [p, w]."""
                dn = pool.tile([p, w], f32, name="sq_dn", tag="sq_dn")
                nc.vector.tensor_scalar_add(dn, n2, 1.0)
                wi = pool.tile([p, w], f32, name="sq_wi", tag="sq_wi")
                nc.vector.reciprocal(wi, dn)
                a = pool.tile([p, w], f32, name="sq_a", tag="sq_a")
                nc.vector.tensor_scalar_add(a, n2, EPS)
                y = rsqrt_dve(pool, a, p, w, iters)
                f = pool.tile([p, w], f32, name="sq_f", tag="sq_f")
                nc.vector.tensor_tensor(f, n2, wi, op=ALU.mult)
                nc.vector.tensor_tensor(f, f, y, op=ALU.mult)
                return f

            # ---- r0 weighted sum: s0 = 0.1 * sum_{i,d} x*W  (all 32 b) ----
            ps0 = psm.tile([BL, CO], f32, tag="pss")
            for kc in range(IG):
                nc.tensor.matmul(ps0, xt_sb[:, kc * BL:(kc + 1) * BL],
                                 w_sb[:, kc * CO:(kc + 1) * CO],
                                 start=(kc == 0), stop=(kc == IG - 1))
            s_all = singles.tile([BL, CO], f32)
            nc.scalar.mul(s_all, ps0, 1.0 / C)

            # r0 squash on [32, CO]: per-(b,c) n2 over o, then scale
            sq32 = singles.tile([BL, CO], f32)
            nc.vector.tensor_mul(sq32, s_all, s_all)
            n2_32 = singles.tile([BL, C], f32)
            nc.vector.tensor_reduce(
                n2_32, sq32.rearrange("p (c o) -> p c o", c=C),
                axis=AX.X, op=ALU.add)
            f32t = squash_scale(sm, n2_32, BL, C, iters=2)
            v0 = singles.tile([BL, CO], f32)
            fb = bass.AP(tensor=f32t.tensor, offset=f32t.offset,
                         ap=[f32t.ap[0], f32t.ap[1], [0, O]])
            nc.vector.tensor_tensor(v0, s_all, fb, op=ALU.mult)
            nc.sync.dma_start(out=vscr_ap, in_=v0)

            st = {}  # per-bt pipeline state

            def emit_einsum(bt):
                ihat = ihp.tile([128, IG * CO], lo, name=f"ihat{bt}", tag="ihat")
                for ch in range(IG // XZ_CHUNK):
                    xz_sb = xzp.tile([128, XZ_CHUNK * 128], lo,
                                     name=f"xz{bt}_{ch}", tag="xz")
                    base = bt * IG + ch * XZ_CHUNK
                    nc.sync.dma_start(
                        out=xz_sb.rearrange("p (t m) -> p t m", t=XZ_CHUNK),
                        in_=xz_ap[base:base + XZ_CHUNK].rearrange(
                            "t p m -> p t m"))
                    for t3 in range(XZ_CHUNK // 3):
                        pih = psum.tile([128, 3 * CO], f32,
                                        name=f"pih{bt}_{ch}_{t3}", tag="pih")
                        for j in range(3):
                            t = t3 * 3 + j
                            ig = ch * XZ_CHUNK + t
                            nc.tensor.matmul(
                                pih[:, j * CO:(j + 1) * CO],
                                xz_sb[:, t * 128:(t + 1) * 128],
                                w_sb[:, ig * CO:(ig + 1) * CO],
                                start=True, stop=True)
                        ig0 = ch * XZ_CHUNK + t3 * 3
                        nc.scalar.copy(
                            ihat[:, ig0 * CO:(ig0 + 3) * CO], pih)
                st[bt] = {"ihat": ihat}

            def emit_h1(bt, r):
                s = st[bt]
                ihat = s["ihat"]
                vrep = vp.tile([128, CO], lo, name=f"vrep{bt}_{r}", tag="vrep")
                if r == 0:
                    vi = bass.AP(tensor=vscr_ap.tensor,
                                 offset=bt * BT * CO,
                                 ap=[[CO, BT], [0, ISUB], [1, CO]])
                else:
                    vsrc = s["v"]
                    vi = bass.AP(tensor=vsrc.tensor, offset=vsrc.offset,
                                 ap=[vsrc.ap[0], [0, ISUB], [1, CO]])
                nc.gpsimd.dma_start(out=vrep, in_=vi)

                delta = dp.tile([128, IG * C], f32, name=f"delta{bt}_{r}", tag="delta")
                GCF = IG * C          # 720 (g,c) pairs, full width
                tc_t = tch.tile([128, IG * CO], lo, name=f"tc{bt}{r}", tag="tc")
                vb = bass.AP(tensor=vrep.tensor, offset=vrep.offset,
                             ap=[vrep.ap[0], [0, IG], [1, CO]])
                nc.vector.tensor_tensor(tc_t, ihat, vb, op=ALU.mult)
                t8 = trp.tile([128, GCF * 8], lo, name=f"t8_{bt}{r}", tag="t8")
                a0 = bass.AP(tensor=tc_t.tensor, offset=tc_t.offset,
                             ap=[tc_t.ap[0], [16, GCF], [1, 8]])
                a1 = bass.AP(tensor=tc_t.tensor, offset=tc_t.offset + 8,
                             ap=[tc_t.ap[0], [16, GCF], [1, 8]])
                d8 = bass.AP(tensor=t8.tensor, offset=t8.offset,
                             ap=[t8.ap[0], [8, GCF], [1, 8]])
                nc.vector.tensor_tensor(d8, a0, a1, op=ALU.add)
                t4 = trp.tile([128, GCF * 4], lo, name=f"t4_{bt}{r}", tag="t4")
                b0 = bass.AP(tensor=t8.tensor, offset=t8.offset,
                             ap=[t8.ap[0], [8, GCF], [1, 4]])
                b1 = bass.AP(tensor=t8.tensor, offset=t8.offset + 4,
                             ap=[t8.ap[0], [8, GCF], [1, 4]])
                d4 = bass.AP(tensor=t4.tensor, offset=t4.offset,
                             ap=[t4.ap[0], [4, GCF], [1, 4]])
                nc.vector.tensor_tensor(d4, b0, b1, op=ALU.add)
                t2 = trp.tile([128, GCF * 2], lo, name=f"t2_{bt}{r}", tag="t2")
                c0 = bass.AP(tensor=t4.tensor, offset=t4.offset,
                             ap=[t4.ap[0], [4, GCF], [1, 2]])
                c1 = bass.AP(tensor=t4.tensor, offset=t4.offset + 2,
                             ap=[t4.ap[0], [4, GCF], [1, 2]])
                d2 = bass.AP(tensor=t2.tensor, offset=t2.offset,
                             ap=[t2.ap[0], [2, GCF], [1, 2]])
                nc.vector.tensor_tensor(d2, c0, c1, op=ALU.add)
                e0 = bass.AP(tensor=t2.tensor, offset=t2.offset,
                             ap=[t2.ap[0], [2, GCF]])
                e1a = bass.AP(tensor=t2.tensor, offset=t2.offset + 1,
                              ap=[t2.ap[0], [2, GCF]])
                nc.vector.tensor_tensor(delta, e0, e1a, op=ALU.add)

                # e = exp(bias), accumulated multiplicatively
                if r == 0:
                    e_t = ep.tile([128, IG * C], lo, name=f"e{bt}", tag="e")
                    nc.scalar.activation(e_t, delta, AF.Exp)
                    s["e"] = e_t
                else:
                    e_t = s["e"]
                    ed = sm.tile([128, IG * C], lo, name=f"ed{bt}", tag="ed")
                    nc.scalar.activation(ed, delta, AF.Exp)
                    nc.vector.tensor_tensor(e_t, e_t, ed, op=ALU.mult)

                zsum = sm.tile([128, IG], f32, name=f"zs{bt}{r}", tag="zs")
                nc.vector.tensor_reduce(
                    zsum, e_t.rearrange("p (g c) -> p g c", c=C),
                    axis=AX.X, op=ALU.add)
                rz = sm.tile([128, IG], f32, name=f"rz{bt}{r}", tag="rz")
                nc.vector.reciprocal(rz, zsum)
                # materialize rz dense (scalar engine) so coup mult is 2x
                rz720 = sm.tile([128, IG * C], lo, name=f"rzm{bt}{r}", tag="rzm")
                rzb = bass.AP(tensor=rz.tensor, offset=rz.offset,
                              ap=[rz.ap[0], [1, IG], [0, C]])
                nc.scalar.copy(rz720, rzb)
                coup = cp.tile([128, IG * C], lo, name=f"coup{bt}{r}", tag="coup")
                nc.vector.tensor_tensor(coup, e_t, rz720, op=ALU.mult)

                # zsc[(b,i),(g,b',c)] = coup[(b,i),(g,c)] * d(b,b')
                zsc = zp.tile([128, IG * BT * C], lo, name=f"zsc{bt}{r}", tag="zsc")
                zr = zsc.rearrange("p (g b c) -> p g b c", b=BT, c=C)
                cb = bass.AP(tensor=coup.tensor, offset=coup.offset,
                             ap=[coup.ap[0], [C, IG], [0, BT], [1, C]])
                mb = bass.AP(tensor=maskz.tensor, offset=maskz.offset,
                             ap=[maskz.ap[0], [0, IG], [C, BT], [1, C]])
                nc.vector.tensor_tensor(zr, cb, mb, op=ALU.mult)
                s["zsc"] = zsc

            def emit_h2(bt, r):
                s = st[bt]
                ihat, zsc = s["ihat"], s["zsc"]
                pss = psm.tile([BT * C, CO], f32, name=f"pss{bt}{r}", tag="pss")
                for ig in range(IG):
                    nc.tensor.matmul(
                        pss, zsc[:, ig * BT * C:(ig + 1) * BT * C],
                        ihat[:, ig * CO:(ig + 1) * CO],
                        start=(ig == 0), stop=(ig == IG - 1))
                sst = sm.tile([BT * C, CO], f32, name=f"sst{bt}{r}", tag="sst")
                nc.vector.tensor_tensor(sst, pss, cmask, op=ALU.mult)
                sjunk = sm.tile([BT * C, CO], f32, name=f"sj{bt}{r}", tag="sj")
                n2_80 = sm.tile([BT * C, 1], f32, name=f"n2{bt}{r}", tag="n2")
                nc.scalar.activation(sjunk, sst, AF.Square,
                                     accum_out=n2_80)
                f80 = squash_scale(sm, n2_80, BT * C, 1,
                                   iters=2 if r == NR - 2 else 1)
                v80 = sm.tile([BT * C, CO], f32, name=f"v80{bt}{r}", tag="v80")
                nc.vector.tensor_scalar_mul(v80, sst, f80)
                v8ps = psm.tile([BT, CO], f32, name=f"v8p{bt}{r}", tag="v8p", bufs=1)
                nc.tensor.matmul(v8ps, sel_sb, v80, start=True, stop=True)
                v_sb = sm.tile([BT, CO], f32, name=f"v{bt}{r}", tag="v")
                nc.scalar.copy(v_sb, v8ps)
                s["v"] = v_sb

            def emit_out(bt):
                nc.sync.dma_start(out=out_ap[bt * BT:(bt + 1) * BT, :],
                                  in_=st[bt]["v"])

            # software-pipelined schedule (see module docstring)
            emit_einsum(0)
            emit_einsum(1)
            emit_h1(0, 0)
            emit_h1(1, 0)
            emit_h2(0, 0)
            emit_h1(0, 1)
            emit_h2(1, 0)
            emit_h2(0, 1)
            emit_out(0)
            emit_einsum(2)
            emit_h1(1, 1)
            emit_h1(2, 0)
            emit_h2(1, 1)
            emit_out(1)
            emit_h2(2, 0)
            emit_einsum(3)
            emit_h1(3, 0)
            emit_h1(2, 1)
            emit_h2(3, 0)
            emit_h2(2, 1)
            emit_out(2)
            emit_h1(3, 1)
            emit_h2(3, 1)
            emit_out(3)

    nc.compile()
    return nc


def _prep_inputs(x, W):
    """Host-side layout transforms (not part of measured HW time)."""
    x = np.ascontiguousarray(x, dtype=F32)
    W = np.ascontiguousarray(W, dtype=F32)
    # W -> [(i_sub, d), (ig, c, o)]
    wr = np.ascontiguousarray(
        W.reshape(IG, ISUB, C, D, O).transpose(1, 3, 0, 2, 4)
    ).reshape(128, IG * CO)

    # x -> per core [core, bt, b, ig, i_sub, d]
    x8 = x.reshape(NCORES, NBT, BT, IG, ISUB, D)

    # block-diagonal lhsT tiles: xz[core, bt, ig, (i_sub,d), (b,i_sub')]
    xz = np.zeros((NCORES, NBT, IG, ISUB, D, 128), dtype=F32)
    isub = np.arange(ISUB)
    for b in range(BT):
        # advanced indexing pulls the i_sub axis to the front
        xz[:, :, :, isub, :, b * ISUB + isub] = \
            x8[:, :, b].transpose(3, 0, 1, 2, 4)
    xz = xz.reshape(NCORES, NBT * IG, 128, 128)

    # compact xT for r0: [core, (i_sub,d), ig, b]
    xt = np.ascontiguousarray(
        x8.reshape(NCORES, BL, IG, ISUB, D).transpose(0, 3, 4, 2, 1)
    ).reshape(NCORES, 128, IG, BL)

    # constants (all (b,c)-ordered partition/row layouts)
    cmask = np.zeros((BT * C, CO), dtype=F32)       # [(b,c'), (c,o)]
    for b in range(BT):
        for c in range(C):
            cmask[b * C + c, c * O:(c + 1) * O] = 1.0
    # maskz[p=(b,i), (b',c)] = 1 iff b' == b
    maskz = np.zeros((128, BT * C), dtype=F32)
    for b in range(BT):
        for c in range(C):
            maskz[b * ISUB:(b + 1) * ISUB, b * C + c] = 1.0
    sel = np.zeros((BT * C, BT), dtype=F32)         # [(b,c'), b2]
    for b in range(BT):
        for c in range(C):
            sel[b * C + c, b] = 1.0

    if USE_BF16:
        from ml_dtypes import bfloat16
        xz = xz.astype(bfloat16)
        xt = xt.astype(bfloat16)
        wr = wr.astype(bfloat16)
        maskz = maskz.astype(bfloat16)
    return xz, xt, wr, cmask, maskz, sel


def kernel(x: np.ndarray, W: np.ndarray) -> np.ndarray:
    from concourse import bass_utils

    if "nc" not in _compiled:
        _compiled["nc"] = _build_program()
    nc = _compiled["nc"]

    xz, xt, wr, cmask, maskz, sel = _prep_inputs(np.asarray(x), np.asarray(W))
    in_maps = [{"xz": xz[c], "xt": xt[c], "w": wr,
                "cmask": cmask, "maskz": maskz, "sel": sel}
               for c in range(NCORES)]
    res = bass_utils.run_bass_kernel_spmd(nc, in_maps, list(range(NCORES)))
    out = np.concatenate([res.results[c]["out"] for c in range(NCORES)], axis=0)
    return out.reshape(B, C, O)


# revision 11
# speedup vs baseline: 1.0841x; 1.0841x over previous
"""CapsuleLayer (dynamic routing) Trainium2 kernel — v3.

Full inputs -> batch-sharded over 8 NeuronCores -> full output.

Math (per sample b):
    ihat[i,c,o] = sum_d x[i,d] * W[i,c,d,o]
    bias = 0
    for r in 0..2:
        coup = softmax(bias, axis=c)
        s[c,o] = sum_i coup[i,c] * ihat[i,c,o]
        v = squash(s)
        if r < 2: bias[i,c] += sum_o ihat[i,c,o] * v[c,o]
    return v

Device layout (per core, 32 local samples, batch-tiles of 8):
    SBUF partition dim p = (b, i_sub): p = b*16 + i_sub, free dim (ig, c, o)
    with ig = i // 16 (72 groups).  ihat tile: [128, 72*10*16] bf16.

v3 perf notes:
  - Exp is the ONLY table-based scalar activation (one ACT_TABLE_LOAD
    total).  rsqrt in squash = bitcast magic-seed + Newton on DVE;
    1/z and 1/(1+n2) via DVE reciprocal.
  - o-reduction of ihat*v: bf16 2x-mode pairwise tree adds.
  - softmax state multiplicative: e *= exp(delta), no f32 bias tensor.
  - zsc coupling lhsT layout (g, b, c), all-bf16 step-1 -> 2x mask mult;
    rz materialized dense by scalar engine so coup mult also runs 2x.
  - einsum PSUM evacuation: 3 ig per 2KB PSUM bank, one scalar copy each.
  - batch tiles software-pipelined: emission order interleaves bt's so
    the DVE stream never waits on PE s-matmuls / scalar exp of the same
    chain; routing state lives in bufs=2 pools, ihat in bufs=3.
"""

import sys

if "/opt/trn_rl_repo" not in sys.path:
    sys.path.insert(0, "/opt/trn_rl_repo")

import numpy as np

B, I, D, C, O = 256, 1152, 8, 10, 16
NCORES = 8
BL = B // NCORES            # 32 local samples per core
NBT, BT = 4, 8              # batch tiles
ISUB = 16                   # i's per group
IG = I // ISUB              # 72 groups
CO = C * O                  # 160
NR = 3
EPS = 1e-7
XZ_CHUNK = 18               # ig's per xz DMA chunk
NCH = 4                     # bias-update chunks
F32 = np.float32
MAGIC = float(0x5F3759DF)   # rsqrt seed magic

USE_BF16 = True

_compiled = {}


def _build_program():
    import concourse.bacc as bacc
    import concourse.tile as tile
    import concourse.mybir as mybir
    import concourse.bass as bass

    f32 = mybir.dt.float32
    i32 = mybir.dt.int32
    lo = mybir.dt.bfloat16 if USE_BF16 else f32
    nc = bacc.Bacc("TRN2", target_bir_lowering=False, debug=False,
                   num_devices=NCORES)

    xz_t = nc.dram_tensor("xz", [NBT * IG, 128, 128], lo, kind="ExternalInput")
    xt_t = nc.dram_tensor("xt", [128, IG, BL], lo, kind="ExternalInput")
    w_t = nc.dram_tensor("w", [128, IG * CO], lo, kind="ExternalInput")
    cmask_t = nc.dram_tensor("cmask", [BT * C, CO], f32, kind="ExternalInput")
    maskz_t = nc.dram_tensor("maskz", [128, BT * C], lo, kind="ExternalInput")
    sel_t = nc.dram_tensor("sel", [BT * C, BT], f32, kind="ExternalInput")
    out_t = nc.dram_tensor("out", [BL, CO], f32, kind="ExternalOutput")
    vscr_t = nc.dram_tensor("vscr", [BL, CO], f32)   # internal scratch
    xz_ap, xt_ap, w_ap = xz_t.ap(), xt_t.ap(), w_t.ap()
    out_ap, vscr_ap = out_t.ap(), vscr_t.ap()

    AF = mybir.ActivationFunctionType
    ALU = mybir.AluOpType
    AX = mybir.AxisListType

    GN = IG // NCH           # 18 groups per chunk
    GC = GN * C              # 180

    with tile.TileContext(nc) as tc:
        from contextlib import ExitStack

        with ExitStack() as ctx:
            singles = ctx.enter_context(tc.tile_pool(name="singles", bufs=1))
            xzp = ctx.enter_context(tc.tile_pool(name="xzp", bufs=2))
            psum = ctx.enter_context(tc.tile_pool(name="psum", bufs=4, space="PSUM"))
            psm = ctx.enter_context(tc.tile_pool(name="psm", bufs=2, space="PSUM"))
            ihp = ctx.enter_context(tc.tile_pool(name="ihp", bufs=3))
            tch = ctx.enter_context(tc.tile_pool(name="tch", bufs=1))
            trp = ctx.enter_context(tc.tile_pool(name="trp", bufs=1))
            dp = ctx.enter_context(tc.tile_pool(name="dp", bufs=2))
            ep = ctx.enter_context(tc.tile_pool(name="ep", bufs=2))
            cp = ctx.enter_context(tc.tile_pool(name="cp", bufs=2))
            zp = ctx.enter_context(tc.tile_pool(name="zp", bufs=2))
            vp = ctx.enter_context(tc.tile_pool(name="vp", bufs=2))
            sm = ctx.enter_context(tc.tile_pool(name="sm", bufs=2))

            w_sb = singles.tile([128, IG * CO], lo)
            nc.sync.dma_start(out=w_sb, in_=w_ap)
            xt_sb = singles.tile([128, IG * BL], lo)
            nc.sync.dma_start(out=xt_sb,
                              in_=xt_ap.rearrange("p g b -> p (g b)"))
            cmask = singles.tile([BT * C, CO], f32)
            nc.sync.dma_start(out=cmask, in_=cmask_t.ap())
            maskz = singles.tile([128, BT * C], lo)
            nc.sync.dma_start(out=maskz, in_=maskz_t.ap())
            sel_sb = singles.tile([BT * C, BT], f32)
            nc.sync.dma_start(out=sel_sb, in_=sel_t.ap())

            def rsqrt_dve(pool, a, p, w, iters):
                """y ~= 1/sqrt(a) on DVE only (magic seed + Newton)."""
                sh = pool.tile([p, w], i32, name="rs_sh", tag="rs_sh")
                nc.vector.tensor_scalar(sh, a.bitcast(i32), 1, None,
                                        op0=ALU.logical_shift_right)
                yi = pool.tile([p, w], i32, name="rs_yi", tag="rs_yi")
                nc.vector.tensor_scalar(yi, sh, -1.0, MAGIC,
                                        op0=ALU.mult, op1=ALU.add)
                y = yi.bitcast(f32)
                for _ in range(iters):
                    t = pool.tile([p, w], f32, name="rs_t", tag="rs_t")
                    nc.vector.tensor_tensor(t, y, y, op=ALU.mult)
                    nc.vector.tensor_tensor(t, t, a, op=ALU.mult)
                    nc.vector.tensor_scalar(t, t, -0.5, 1.5,
                                            op0=ALU.mult, op1=ALU.add)
                    yn = pool.tile([p, w], f32, name="rs_yn", tag="rs_yn")
                    nc.vector.tensor_tensor(yn, y, t, op=ALU.mult)
                    y = yn
                return y

            def squash_scale(pool, n2, p, w, iters):
                """f = n2 / ((1+n2)*sqrt(n2+eps)), DVE only, [p, w]."""
                dn = pool.tile([p, w], f32, name="sq_dn", tag="sq_dn")
                nc.vector.tensor_scalar_add(dn, n2, 1.0)
                wi = pool.tile([p, w], f32, name="sq_wi", tag="sq_wi")
                nc.vector.reciprocal(wi, dn)
                a = pool.tile([p, w], f32, name="sq_a", tag="sq_a")
                nc.vector.tensor_scalar_add(a, n2, EPS)
                y = rsqrt_dve(pool, a, p, w, iters)
                f = pool.tile([p, w], f32, name="sq_f", tag="sq_f")
                nc.vector.tensor_tensor(f, n2, wi, op=ALU.mult)
                nc.vector.tensor_tensor(f, f, y, op=ALU.mult)
                return f

            # ---- r0 weighted sum: s0 = 0.1 * sum_{i,d} x*W  (all 32 b) ----
            ps0 = psm.tile([BL, CO], f32, tag="pss")
            for kc in range(IG):
                nc.tensor.matmul(ps0, xt_sb[:, kc * BL:(kc + 1) * BL],
                                 w_sb[:, kc * CO:(kc + 1) * CO],
                                 start=(kc == 0), stop=(kc == IG - 1))
            s_all = singles.tile([BL, CO], f32)
            nc.scalar.mul(s_all, ps0, 1.0 / C)

            # r0 squash on [32, CO]: per-(b,c) n2 over o, then scale
            sq32 = singles.tile([BL, CO], f32)
            nc.vector.tensor_mul(sq32, s_all, s_all)
            n2_32 = singles.tile([BL, C], f32)
            nc.vector.tensor_reduce(
                n2_32, sq32.rearrange("p (c o) -> p c o", c=C),
                axis=AX.X, op=ALU.add)
            f32t = squash_scale(sm, n2_32, BL, C, iters=2)
            v0 = singles.tile([BL, CO], f32)
            fb = bass.AP(tensor=f32t.tensor, offset=f32t.offset,
                         ap=[f32t.ap[0], f32t.ap[1], [0, O]])
            nc.vector.tensor_tensor(v0, s_all, fb, op=ALU.mult)
            nc.sync.dma_start(out=vscr_ap, in_=v0)

            st = {}  # per-bt pipeline state

            def emit_einsum(bt):
                ihat = ihp.tile([128, IG * CO], lo, name=f"ihat{bt}", tag="ihat")
                for ch in range(IG // XZ_CHUNK):
                    xz_sb = xzp.tile([128, XZ_CHUNK * 128], lo,
                                     name=f"xz{bt}_{ch}", tag="xz")
                    base = bt * IG + ch * XZ_CHUNK
                    nc.sync.dma_start(
                        out=xz_sb.rearrange("p (t m) -> p t m", t=XZ_CHUNK),
                        in_=xz_ap[base:base + XZ_CHUNK].rearrange(
                            "t p m -> p t m"))
                    for t3 in range(XZ_CHUNK // 3):
                        pih = psum.tile([128, 3 * CO], f32,
                                        name=f"pih{bt}_{ch}_{t3}", tag="pih")
                        for j in range(3):
                            t = t3 * 3 + j
                            ig = ch * XZ_CHUNK + t
                            nc.tensor.matmul(
                                pih[:, j * CO:(j + 1) * CO],
                                xz_sb[:, t * 128:(t + 1) * 128],
                                w_sb[:, ig * CO:(ig + 1) * CO],
                                start=True, stop=True)
                        ig0 = ch * XZ_CHUNK + t3 * 3
                        nc.scalar.copy(
                            ihat[:, ig0 * CO:(ig0 + 3) * CO], pih)
                st[bt] = {"ihat": ihat}

            def emit_h1(bt, r):
                s = st[bt]
                ihat = s["ihat"]
                vrep = vp.tile([128, CO], lo, name=f"vrep{bt}_{r}", tag="vrep")
                if r == 0:
                    vi = bass.AP(tensor=vscr_ap.tensor,
                                 offset=bt * BT * CO,
                                 ap=[[CO, BT], [0, ISUB], [1, CO]])
                else:
                    vsrc = s["v"]
                    vi = bass.AP(tensor=vsrc.tensor, offset=vsrc.offset,
                                 ap=[vsrc.ap[0], [0, ISUB], [1, CO]])
                nc.gpsimd.dma_start(out=vrep, in_=vi)

                delta = dp.tile([128, IG * C], f32, name=f"delta{bt}_{r}", tag="delta")
                GCF = IG * C          # 720 (g,c) pairs, full width
                tc_t = tch.tile([128, IG * CO], lo, name=f"tc{bt}{r}", tag="tc")
                vb = bass.AP(tensor=vrep.tensor, offset=vrep.offset,
                             ap=[vrep.ap[0], [0, IG], [1, CO]])
                nc.vector.tensor_tensor(tc_t, ihat, vb, op=ALU.mult)
                t8 = trp.tile([128, GCF * 8], lo, name=f"t8_{bt}{r}", tag="t8")
                a0 = bass.AP(tensor=tc_t.tensor, offset=tc_t.offset,
                             ap=[tc_t.ap[0], [16, GCF], [1, 8]])
                a1 = bass.AP(tensor=tc_t.tensor, offset=tc_t.offset + 8,
                             ap=[tc_t.ap[0], [16, GCF], [1, 8]])
                d8 = bass.AP(tensor=t8.tensor, offset=t8.offset,
                             ap=[t8.ap[0], [8, GCF], [1, 8]])
                nc.vector.tensor_tensor(d8, a0, a1, op=ALU.add)
                t4 = trp.tile([128, GCF * 4], lo, name=f"t4_{bt}{r}", tag="t4")
                b0 = bass.AP(tensor=t8.tensor, offset=t8.offset,
                             ap=[t8.ap[0], [8, GCF], [1, 4]])
                b1 = bass.AP(tensor=t8.tensor, offset=t8.offset + 4,
                             ap=[t8.ap[0], [8, GCF], [1, 4]])
                d4 = bass.AP(tensor=t4.tensor, offset=t4.offset,
                             ap=[t4.ap[0], [4, GCF], [1, 4]])
                nc.vector.tensor_tensor(d4, b0, b1, op=ALU.add)
                t2 = trp.tile([128, GCF * 2], lo, name=f"t2_{bt}{r}", tag="t2")
                c0 = bass.AP(tensor=t4.tensor, offset=t4.offset,
                             ap=[t4.ap[0], [4, GCF], [1, 2]])
                c1 = bass.AP(tensor=t4.tensor, offset=t4.offset + 2,
                             ap=[t4.ap[0], [4, GCF], [1, 2]])
                d2 = bass.AP(tensor=t2.tensor, offset=t2.offset,
                             ap=[t2.ap[0], [2, GCF], [1, 2]])
                nc.vector.tensor_tensor(d2, c0, c1, op=ALU.add)
                e0 = bass.AP(tensor=t2.tensor, offset=t2.offset,
                             ap=[t2.ap[0], [2, GCF]])
                e1a = bass.AP(tensor=t2.tensor, offset=t2.offset + 1,
                              ap=[t2.ap[0], [2, GCF]])
                nc.vector.tensor_tensor(delta, e0, e1a, op=ALU.add)

                # e = exp(bias), accumulated multiplicatively
                if r == 0:
                    e_t = ep.tile([128, IG * C], lo, name=f"e{bt}", tag="e")
                    nc.scalar.activation(e_t, delta, AF.Exp)
                    s["e"] = e_t
                else:
                    e_t = s["e"]
                    ed = sm.tile([128, IG * C], lo, name=f"ed{bt}", tag="ed")
                    nc.scalar.activation(ed, delta, AF.Exp)
                    nc.vector.tensor_tensor(e_t, e_t, ed, op=ALU.mult)

                zsum = sm.tile([128, IG], f32, name=f"zs{bt}{r}", tag="zs")
                nc.vector.tensor_reduce(
                    zsum, e_t.rearrange("p (g c) -> p g c", c=C),
                    axis=AX.X, op=ALU.add)
                rz = sm.tile([128, IG], f32, name=f"rz{bt}{r}", tag="rz")
                nc.vector.reciprocal(rz, zsum)
                # materialize rz dense (scalar engine) so coup mult is 2x
                rz720 = sm.tile([128, IG * C], lo, name=f"rzm{bt}{r}", tag="rzm")
                rzb = bass.AP(tensor=rz.tensor, offset=rz.offset,
                              ap=[rz.ap[0], [1, IG], [0, C]])
                nc.scalar.copy(rz720, rzb)
                coup = cp.tile([128, IG * C], lo, name=f"coup{bt}{r}", tag="coup")
                nc.vector.tensor_tensor(coup, e_t, rz720, op=ALU.mult)

                # zsc[(b,i),(g,b',c)] = coup[(b,i),(g,c)] * d(b,b')
                zsc = zp.tile([128, IG * BT * C], lo, name=f"zsc{bt}{r}", tag="zsc")
                zr = zsc.rearrange("p (g b c) -> p g b c", b=BT, c=C)
                cb = bass.AP(tensor=coup.tensor, offset=coup.offset,
                             ap=[coup.ap[0], [C, IG], [0, BT], [1, C]])
                mb = bass.AP(tensor=maskz.tensor, offset=maskz.offset,
                             ap=[maskz.ap[0], [0, IG], [C, BT], [1, C]])
                nc.vector.tensor_tensor(zr, cb, mb, op=ALU.mult)
                s["zsc"] = zsc

            def emit_h2(bt, r):
                s = st[bt]
                ihat, zsc = s["ihat"], s["zsc"]
                pss = psm.tile([BT * C, CO], f32, name=f"pss{bt}{r}", tag="pss")
                for ig in range(IG):
                    nc.tensor.matmul(
                        pss, zsc[:, ig * BT * C:(ig + 1) * BT * C],
                        ihat[:, ig * CO:(ig + 1) * CO],
                        start=(ig == 0), stop=(ig == IG - 1))
                sst = sm.tile([BT * C, CO], f32, name=f"sst{bt}{r}", tag="sst")
                nc.vector.tensor_tensor(sst, pss, cmask, op=ALU.mult)
                sjunk = sm.tile([BT * C, CO], f32, name=f"sj{bt}{r}", tag="sj")
                n2_80 = sm.tile([BT * C, 1], f32, name=f"n2{bt}{r}", tag="n2")
                nc.scalar.activation(sjunk, sst, AF.Square,
                                     accum_out=n2_80)
                f80 = squash_scale(sm, n2_80, BT * C, 1,
                                   iters=2 if r == NR - 2 else 1)
                v80 = sm.tile([BT * C, CO], f32, name=f"v80{bt}{r}", tag="v80")
                nc.vector.tensor_scalar_mul(v80, sst, f80)
                v8ps = psm.tile([BT, CO], f32, name=f"v8p{bt}{r}", tag="v8p", bufs=1)
                nc.tensor.matmul(v8ps, sel_sb, v80, start=True, stop=True)
                v_sb = sm.tile([BT, CO], f32, name=f"v{bt}{r}", tag="v")
                nc.scalar.copy(v_sb, v8ps)
                s["v"] = v_sb

            def emit_out(bt):
                nc.sync.dma_start(out=out_ap[bt * BT:(bt + 1) * BT, :],
                                  in_=st[bt]["v"])

            # software-pipelined schedule (see module docstring)
            emit_einsum(0)
            emit_einsum(1)
            emit_h1(0, 0)
            emit_h1(1, 0)
            emit_h2(0, 0)
            emit_h1(0, 1)
            emit_h2(1, 0)
            emit_h2(0, 1)
            emit_out(0)
            emit_einsum(2)
            emit_h1(1, 1)
            emit_h1(2, 0)
            emit_h2(1, 1)
            emit_out(1)
            emit_h2(2, 0)
            emit_einsum(3)
            emit_h1(3, 0)
            emit_h1(2, 1)
            emit_h2(3, 0)
            emit_h2(2, 1)
            emit_out(2)
            emit_h1(3, 1)
            emit_h2(3, 1)
            emit_out(3)

    nc.compile()
    return nc


def _prep_inputs(x, W):
    """Host-side layout transforms (not part of measured HW time)."""
    x = np.ascontiguousarray(x, dtype=F32)
    W = np.ascontiguousarray(W, dtype=F32)
    # W -> [(i_sub, d), (ig, c, o)]
    wr = np.ascontiguousarray(
        W.reshape(IG, ISUB, C, D, O).transpose(1, 3, 0, 2, 4)
    ).reshape(128, IG * CO)

    # x -> per core [core, bt, b, ig, i_sub, d]
    x8 = x.reshape(NCORES, NBT, BT, IG, ISUB, D)

    # block-diagonal lhsT tiles: xz[core, bt, ig, (i_sub,d), (b,i_sub')]
    xz = np.zeros((NCORES, NBT, IG, ISUB, D, 128), dtype=F32)
    isub = np.arange(ISUB)
    for b in range(BT):
        # advanced indexing pulls the i_sub axis to the front
        xz[:, :, :, isub, :, b * ISUB + isub] = \
            x8[:, :, b].transpose(3, 0, 1, 2, 4)
    xz = xz.reshape(NCORES, NBT * IG, 128, 128)

    # compact xT for r0: [core, (i_sub,d), ig, b]
    xt = np.ascontiguousarray(
        x8.reshape(NCORES, BL, IG, ISUB, D).transpose(0, 3, 4, 2, 1)
    ).reshape(NCORES, 128, IG, BL)

    # constants (all (b,c)-ordered partition/row layouts)
    cmask = np.zeros((BT * C, CO), dtype=F32)       # [(b,c'), (c,o)]
    for b in range(BT):
        for c in range(C):
            cmask[b * C + c, c * O:(c + 1) * O] = 1.0
    # maskz[p=(b,i), (b',c)] = 1 iff b' == b
    maskz = np.zeros((128, BT * C), dtype=F32)
    for b in range(BT):
        for c in range(C):
            maskz[b * ISUB:(b + 1) * ISUB, b * C + c] = 1.0
    sel = np.zeros((BT * C, BT), dtype=F32)         # [(b,c'), b2]
    for b in range(BT):
        for c in range(C):
            sel[b * C + c, b] = 1.0

    if USE_BF16:
        from ml_dtypes import bfloat16
        xz = xz.astype(bfloat16)
        xt = xt.astype(bfloat16)
        wr = wr.astype(bfloat16)
        maskz = maskz.astype(bfloat16)
    return xz, xt, wr, cmask, maskz, sel


def kernel(x: np.ndarray, W: np.ndarray) -> np.ndarray:
    from concourse import bass_utils

    if "nc" not in _compiled:
        _compiled["nc"] = _build_program()
    nc = _compiled["nc"]

    xz, xt, wr, cmask, maskz, sel = _prep_inputs(np.asarray(x), np.asarray(W))
    in_maps = [{"xz": xz[c], "xt": xt[c], "w": wr,
                "cmask": cmask, "maskz": maskz, "sel": sel}
               for c in range(NCORES)]
    res = bass_utils.run_bass_kernel_spmd(nc, in_maps, list(range(NCORES)))
    out = np.concatenate([res.results[c]["out"] for c in range(NCORES)], axis=0)
    return out.reshape(B, C, O)


# revision 12
# speedup vs baseline: 1.1003x; 1.0150x over previous
"""CapsuleLayer (dynamic routing) Trainium2 kernel — v3.

Full inputs -> batch-sharded over 8 NeuronCores -> full output.

Math (per sample b):
    ihat[i,c,o] = sum_d x[i,d] * W[i,c,d,o]
    bias = 0
    for r in 0..2:
        coup = softmax(bias, axis=c)
        s[c,o] = sum_i coup[i,c] * ihat[i,c,o]
        v = squash(s)
        if r < 2: bias[i,c] += sum_o ihat[i,c,o] * v[c,o]
    return v

Device layout (per core, 32 local samples, batch-tiles of 8):
    SBUF partition dim p = (b, i_sub): p = b*16 + i_sub, free dim (ig, c, o)
    with ig = i // 16 (72 groups).  ihat tile: [128, 72*10*16] bf16.

v3 perf notes:
  - Exp is the ONLY table-based scalar activation (one ACT_TABLE_LOAD
    total).  rsqrt in squash = bitcast magic-seed + Newton on DVE;
    1/z and 1/(1+n2) via DVE reciprocal.
  - o-reduction of ihat*v: bf16 2x-mode pairwise tree adds.
  - softmax state multiplicative: e *= exp(delta), no f32 bias tensor.
  - zsc coupling lhsT layout (g, b, c), all-bf16 step-1 -> 2x mask mult;
    rz materialized dense by scalar engine so coup mult also runs 2x.
  - einsum PSUM evacuation: 3 ig per 2KB PSUM bank, one scalar copy each.
  - batch tiles software-pipelined: emission order interleaves bt's so
    the DVE stream never waits on PE s-matmuls / scalar exp of the same
    chain; routing state lives in bufs=2 pools, ihat in bufs=3.
"""

import sys

if "/opt/trn_rl_repo" not in sys.path:
    sys.path.insert(0, "/opt/trn_rl_repo")

import numpy as np

B, I, D, C, O = 256, 1152, 8, 10, 16
NCORES = 8
BL = B // NCORES            # 32 local samples per core
NBT, BT = 4, 8              # batch tiles
ISUB = 16                   # i's per group
IG = I // ISUB              # 72 groups
CO = C * O                  # 160
NR = 3
EPS = 1e-7
XZ_CHUNK = 18               # ig's per xz DMA chunk
NCH = 4                     # bias-update chunks
F32 = np.float32
MAGIC = float(0x5F3759DF)   # rsqrt seed magic

USE_BF16 = True

_compiled = {}


def _build_program():
    import concourse.bacc as bacc
    import concourse.tile as tile
    import concourse.mybir as mybir
    import concourse.bass as bass

    f32 = mybir.dt.float32
    i32 = mybir.dt.int32
    lo = mybir.dt.bfloat16 if USE_BF16 else f32
    nc = bacc.Bacc("TRN2", target_bir_lowering=False, debug=False,
                   num_devices=NCORES)

    xz_t = nc.dram_tensor("xz", [NBT * IG, 128, 128], lo, kind="ExternalInput")
    xt_t = nc.dram_tensor("xt", [128, IG, BL], lo, kind="ExternalInput")
    w_t = nc.dram_tensor("w", [128, IG * CO], lo, kind="ExternalInput")
    cmask_t = nc.dram_tensor("cmask", [BT * C, CO], f32, kind="ExternalInput")
    maskz_t = nc.dram_tensor("maskz", [128, BT * C], lo, kind="ExternalInput")
    sel_t = nc.dram_tensor("sel", [BT * C, BT], f32, kind="ExternalInput")
    out_t = nc.dram_tensor("out", [BL, CO], f32, kind="ExternalOutput")
    vscr_t = nc.dram_tensor("vscr", [BL, CO], f32)   # internal scratch
    xz_ap, xt_ap, w_ap = xz_t.ap(), xt_t.ap(), w_t.ap()
    out_ap, vscr_ap = out_t.ap(), vscr_t.ap()

    AF = mybir.ActivationFunctionType
    ALU = mybir.AluOpType
    AX = mybir.AxisListType

    GN = IG // NCH           # 18 groups per chunk
    GC = GN * C              # 180

    with tile.TileContext(nc) as tc:
        from contextlib import ExitStack

        with ExitStack() as ctx:
            singles = ctx.enter_context(tc.tile_pool(name="singles", bufs=1))
            xzp = ctx.enter_context(tc.tile_pool(name="xzp", bufs=3))
            psum = ctx.enter_context(tc.tile_pool(name="psum", bufs=4, space="PSUM"))
            psm = ctx.enter_context(tc.tile_pool(name="psm", bufs=2, space="PSUM"))
            ihp = ctx.enter_context(tc.tile_pool(name="ihp", bufs=3))
            tch = ctx.enter_context(tc.tile_pool(name="tch", bufs=1))
            trp = ctx.enter_context(tc.tile_pool(name="trp", bufs=1))
            dp = ctx.enter_context(tc.tile_pool(name="dp", bufs=2))
            ep = ctx.enter_context(tc.tile_pool(name="ep", bufs=2))
            cp = ctx.enter_context(tc.tile_pool(name="cp", bufs=2))
            zp = ctx.enter_context(tc.tile_pool(name="zp", bufs=2))
            vp = ctx.enter_context(tc.tile_pool(name="vp", bufs=2))
            sm = ctx.enter_context(tc.tile_pool(name="sm", bufs=2))

            w_sb = singles.tile([128, IG * CO], lo)
            nc.sync.dma_start(out=w_sb, in_=w_ap)
            xt_sb = singles.tile([128, IG * BL], lo)
            nc.sync.dma_start(out=xt_sb,
                              in_=xt_ap.rearrange("p g b -> p (g b)"))
            cmask = singles.tile([BT * C, CO], f32)
            nc.sync.dma_start(out=cmask, in_=cmask_t.ap())
            maskz = singles.tile([128, BT * C], lo)
            nc.sync.dma_start(out=maskz, in_=maskz_t.ap())
            sel_sb = singles.tile([BT * C, BT], f32)
            nc.sync.dma_start(out=sel_sb, in_=sel_t.ap())

            def rsqrt_dve(pool, a, p, w, iters):
                """y ~= 1/sqrt(a) on DVE only (magic seed + Newton)."""
                sh = pool.tile([p, w], i32, name="rs_sh", tag="rs_sh")
                nc.vector.tensor_scalar(sh, a.bitcast(i32), 1, None,
                                        op0=ALU.logical_shift_right)
                yi = pool.tile([p, w], i32, name="rs_yi", tag="rs_yi")
                nc.vector.tensor_scalar(yi, sh, -1.0, MAGIC,
                                        op0=ALU.mult, op1=ALU.add)
                y = yi.bitcast(f32)
                for _ in range(iters):
                    t = pool.tile([p, w], f32, name="rs_t", tag="rs_t")
                    nc.vector.tensor_tensor(t, y, y, op=ALU.mult)
                    nc.vector.tensor_tensor(t, t, a, op=ALU.mult)
                    nc.vector.tensor_scalar(t, t, -0.5, 1.5,
                                            op0=ALU.mult, op1=ALU.add)
                    yn = pool.tile([p, w], f32, name="rs_yn", tag="rs_yn")
                    nc.vector.tensor_tensor(yn, y, t, op=ALU.mult)
                    y = yn
                return y

            def squash_scale(pool, n2, p, w, iters):
                """f = n2 / ((1+n2)*sqrt(n2+eps)), DVE only, [p, w]."""
                dn = pool.tile([p, w], f32, name="sq_dn", tag="sq_dn")
                nc.vector.tensor_scalar_add(dn, n2, 1.0)
                wi = pool.tile([p, w], f32, name="sq_wi", tag="sq_wi")
                nc.vector.reciprocal(wi, dn)
                a = pool.tile([p, w], f32, name="sq_a", tag="sq_a")
                nc.vector.tensor_scalar_add(a, n2, EPS)
                y = rsqrt_dve(pool, a, p, w, iters)
                f = pool.tile([p, w], f32, name="sq_f", tag="sq_f")
                nc.vector.tensor_tensor(f, n2, wi, op=ALU.mult)
                nc.vector.tensor_tensor(f, f, y, op=ALU.mult)
                return f

            # ---- r0 weighted sum: s0 = 0.1 * sum_{i,d} x*W  (all 32 b) ----
            ps0 = psm.tile([BL, CO], f32, tag="pss")
            for kc in range(IG):
                nc.tensor.matmul(ps0, xt_sb[:, kc * BL:(kc + 1) * BL],
                                 w_sb[:, kc * CO:(kc + 1) * CO],
                                 start=(kc == 0), stop=(kc == IG - 1))
            s_all = singles.tile([BL, CO], f32)
            nc.scalar.mul(s_all, ps0, 1.0 / C)

            # r0 squash on [32, CO]: per-(b,c) n2 over o, then scale
            sq32 = singles.tile([BL, CO], f32)
            nc.vector.tensor_mul(sq32, s_all, s_all)
            n2_32 = singles.tile([BL, C], f32)
            nc.vector.tensor_reduce(
                n2_32, sq32.rearrange("p (c o) -> p c o", c=C),
                axis=AX.X, op=ALU.add)
            f32t = squash_scale(sm, n2_32, BL, C, iters=2)
            v0 = singles.tile([BL, CO], f32)
            fb = bass.AP(tensor=f32t.tensor, offset=f32t.offset,
                         ap=[f32t.ap[0], f32t.ap[1], [0, O]])
            nc.vector.tensor_tensor(v0, s_all, fb, op=ALU.mult)
            nc.sync.dma_start(out=vscr_ap, in_=v0)

            st = {}  # per-bt pipeline state

            def emit_einsum(bt):
                ihat = ihp.tile([128, IG * CO], lo, name=f"ihat{bt}", tag="ihat")
                for ch in range(IG // XZ_CHUNK):
                    xz_sb = xzp.tile([128, XZ_CHUNK * 128], lo,
                                     name=f"xz{bt}_{ch}", tag="xz")
                    base = bt * IG + ch * XZ_CHUNK
                    nc.sync.dma_start(
                        out=xz_sb.rearrange("p (t m) -> p t m", t=XZ_CHUNK),
                        in_=xz_ap[base:base + XZ_CHUNK].rearrange(
                            "t p m -> p t m"))
                    for t3 in range(XZ_CHUNK // 3):
                        pih = psum.tile([128, 3 * CO], f32,
                                        name=f"pih{bt}_{ch}_{t3}", tag="pih")
                        for j in range(3):
                            t = t3 * 3 + j
                            ig = ch * XZ_CHUNK + t
                            nc.tensor.matmul(
                                pih[:, j * CO:(j + 1) * CO],
                                xz_sb[:, t * 128:(t + 1) * 128],
                                w_sb[:, ig * CO:(ig + 1) * CO],
                                start=True, stop=True)
                        ig0 = ch * XZ_CHUNK + t3 * 3
                        nc.scalar.copy(
                            ihat[:, ig0 * CO:(ig0 + 3) * CO], pih)
                st[bt] = {"ihat": ihat}

            def emit_h1(bt, r):
                s = st[bt]
                ihat = s["ihat"]
                vrep = vp.tile([128, CO], lo, name=f"vrep{bt}_{r}", tag="vrep")
                if r == 0:
                    vi = bass.AP(tensor=vscr_ap.tensor,
                                 offset=bt * BT * CO,
                                 ap=[[CO, BT], [0, ISUB], [1, CO]])
                else:
                    vsrc = s["v"]
                    vi = bass.AP(tensor=vsrc.tensor, offset=vsrc.offset,
                                 ap=[vsrc.ap[0], [0, ISUB], [1, CO]])
                nc.gpsimd.dma_start(out=vrep, in_=vi)

                delta = dp.tile([128, IG * C], f32, name=f"delta{bt}_{r}", tag="delta")
                GCF = IG * C          # 720 (g,c) pairs, full width
                tc_t = tch.tile([128, IG * CO], lo, name=f"tc{bt}{r}", tag="tc")
                vb = bass.AP(tensor=vrep.tensor, offset=vrep.offset,
                             ap=[vrep.ap[0], [0, IG], [1, CO]])
                nc.vector.tensor_tensor(tc_t, ihat, vb, op=ALU.mult)
                t8 = trp.tile([128, GCF * 8], lo, name=f"t8_{bt}{r}", tag="t8")
                a0 = bass.AP(tensor=tc_t.tensor, offset=tc_t.offset,
                             ap=[tc_t.ap[0], [16, GCF], [1, 8]])
                a1 = bass.AP(tensor=tc_t.tensor, offset=tc_t.offset + 8,
                             ap=[tc_t.ap[0], [16, GCF], [1, 8]])
                d8 = bass.AP(tensor=t8.tensor, offset=t8.offset,
                             ap=[t8.ap[0], [8, GCF], [1, 8]])
                nc.vector.tensor_tensor(d8, a0, a1, op=ALU.add)
                t4 = trp.tile([128, GCF * 4], lo, name=f"t4_{bt}{r}", tag="t4")
                b0 = bass.AP(tensor=t8.tensor, offset=t8.offset,
                             ap=[t8.ap[0], [8, GCF], [1, 4]])
                b1 = bass.AP(tensor=t8.tensor, offset=t8.offset + 4,
                             ap=[t8.ap[0], [8, GCF], [1, 4]])
                d4 = bass.AP(tensor=t4.tensor, offset=t4.offset,
                             ap=[t4.ap[0], [4, GCF], [1, 4]])
                nc.vector.tensor_tensor(d4, b0, b1, op=ALU.add)
                t2 = trp.tile([128, GCF * 2], lo, name=f"t2_{bt}{r}", tag="t2")
                c0 = bass.AP(tensor=t4.tensor, offset=t4.offset,
                             ap=[t4.ap[0], [4, GCF], [1, 2]])
                c1 = bass.AP(tensor=t4.tensor, offset=t4.offset + 2,
                             ap=[t4.ap[0], [4, GCF], [1, 2]])
                d2 = bass.AP(tensor=t2.tensor, offset=t2.offset,
                             ap=[t2.ap[0], [2, GCF], [1, 2]])
                nc.vector.tensor_tensor(d2, c0, c1, op=ALU.add)
                e0 = bass.AP(tensor=t2.tensor, offset=t2.offset,
                             ap=[t2.ap[0], [2, GCF]])
                e1a = bass.AP(tensor=t2.tensor, offset=t2.offset + 1,
                              ap=[t2.ap[0], [2, GCF]])
                nc.vector.tensor_tensor(delta, e0, e1a, op=ALU.add)

                # e = exp(bias), accumulated multiplicatively
                if r == 0:
                    e_t = ep.tile([128, IG * C], lo, name=f"e{bt}", tag="e")
                    nc.scalar.activation(e_t, delta, AF.Exp)
                    s["e"] = e_t
                else:
                    e_t = s["e"]
                    ed = sm.tile([128, IG * C], lo, name=f"ed{bt}", tag="ed")
                    nc.scalar.activation(ed, delta, AF.Exp)
                    nc.vector.tensor_tensor(e_t, e_t, ed, op=ALU.mult)

                zsum = sm.tile([128, IG], f32, name=f"zs{bt}{r}", tag="zs")
                nc.vector.tensor_reduce(
                    zsum, e_t.rearrange("p (g c) -> p g c", c=C),
                    axis=AX.X, op=ALU.add)
                rz = sm.tile([128, IG], f32, name=f"rz{bt}{r}", tag="rz")
                nc.vector.reciprocal(rz, zsum)
                # materialize rz dense (scalar engine) so coup mult is 2x
                rz720 = sm.tile([128, IG * C], lo, name=f"rzm{bt}{r}", tag="rzm")
                rzb = bass.AP(tensor=rz.tensor, offset=rz.offset,
                              ap=[rz.ap[0], [1, IG], [0, C]])
                nc.scalar.copy(rz720, rzb)
                coup = cp.tile([128, IG * C], lo, name=f"coup{bt}{r}", tag="coup")
                nc.vector.tensor_tensor(coup, e_t, rz720, op=ALU.mult)

                # zsc[(b,i),(g,b',c)] = coup[(b,i),(g,c)] * d(b,b')
                zsc = zp.tile([128, IG * BT * C], lo, name=f"zsc{bt}{r}", tag="zsc")
                zr = zsc.rearrange("p (g b c) -> p g b c", b=BT, c=C)
                cb = bass.AP(tensor=coup.tensor, offset=coup.offset,
                             ap=[coup.ap[0], [C, IG], [0, BT], [1, C]])
                mb = bass.AP(tensor=maskz.tensor, offset=maskz.offset,
                             ap=[maskz.ap[0], [0, IG], [C, BT], [1, C]])
                nc.vector.tensor_tensor(zr, cb, mb, op=ALU.mult)
                s["zsc"] = zsc

            def emit_h2(bt, r):
                s = st[bt]
                ihat, zsc = s["ihat"], s["zsc"]
                pss = psm.tile([BT * C, CO], f32, name=f"pss{bt}{r}", tag="pss")
                for ig in range(IG):
                    nc.tensor.matmul(
                        pss, zsc[:, ig * BT * C:(ig + 1) * BT * C],
                        ihat[:, ig * CO:(ig + 1) * CO],
                        start=(ig == 0), stop=(ig == IG - 1))
                sst = sm.tile([BT * C, CO], f32, name=f"sst{bt}{r}", tag="sst")
                nc.vector.tensor_tensor(sst, pss, cmask, op=ALU.mult)
                sjunk = sm.tile([BT * C, CO], f32, name=f"sj{bt}{r}", tag="sj")
                n2_80 = sm.tile([BT * C, 1], f32, name=f"n2{bt}{r}", tag="n2")
                nc.scalar.activation(sjunk, sst, AF.Square,
                                     accum_out=n2_80)
                f80 = squash_scale(sm, n2_80, BT * C, 1,
                                   iters=2 if r == NR - 2 else 1)
                v80 = sm.tile([BT * C, CO], f32, name=f"v80{bt}{r}", tag="v80")
                nc.vector.tensor_scalar_mul(v80, sst, f80)
                v8ps = psm.tile([BT, CO], f32, name=f"v8p{bt}{r}", tag="v8p", bufs=1)
                nc.tensor.matmul(v8ps, sel_sb, v80, start=True, stop=True)
                v_sb = sm.tile([BT, CO], f32, name=f"v{bt}{r}", tag="v")
                nc.scalar.copy(v_sb, v8ps)
                s["v"] = v_sb

            def emit_out(bt):
                nc.sync.dma_start(out=out_ap[bt * BT:(bt + 1) * BT, :],
                                  in_=st[bt]["v"])

            # software-pipelined schedule (see module docstring)
            emit_einsum(0)
            emit_einsum(1)
            emit_h1(0, 0)
            emit_h1(1, 0)
            emit_h2(0, 0)
            emit_h1(0, 1)
            emit_h2(1, 0)
            emit_h2(0, 1)
            emit_out(0)
            emit_einsum(2)
            emit_h1(1, 1)
            emit_h1(2, 0)
            emit_h2(1, 1)
            emit_out(1)
            emit_h2(2, 0)
            emit_einsum(3)
            emit_h1(3, 0)
            emit_h1(2, 1)
            emit_h2(3, 0)
            emit_h2(2, 1)
            emit_out(2)
            emit_h1(3, 1)
            emit_h2(3, 1)
            emit_out(3)

    nc.compile()
    return nc


def _prep_inputs(x, W):
    """Host-side layout transforms (not part of measured HW time)."""
    x = np.ascontiguousarray(x, dtype=F32)
    W = np.ascontiguousarray(W, dtype=F32)
    # W -> [(i_sub, d), (ig, c, o)]
    wr = np.ascontiguousarray(
        W.reshape(IG, ISUB, C, D, O).transpose(1, 3, 0, 2, 4)
    ).reshape(128, IG * CO)

    # x -> per core [core, bt, b, ig, i_sub, d]
    x8 = x.reshape(NCORES, NBT, BT, IG, ISUB, D)

    # block-diagonal lhsT tiles: xz[core, bt, ig, (i_sub,d), (b,i_sub')]
    xz = np.zeros((NCORES, NBT, IG, ISUB, D, 128), dtype=F32)
    isub = np.arange(ISUB)
    for b in range(BT):
        # advanced indexing pulls the i_sub axis to the front
        xz[:, :, :, isub, :, b * ISUB + isub] = \
            x8[:, :, b].transpose(3, 0, 1, 2, 4)
    xz = xz.reshape(NCORES, NBT * IG, 128, 128)

    # compact xT for r0: [core, (i_sub,d), ig, b]
    xt = np.ascontiguousarray(
        x8.reshape(NCORES, BL, IG, ISUB, D).transpose(0, 3, 4, 2, 1)
    ).reshape(NCORES, 128, IG, BL)

    # constants (all (b,c)-ordered partition/row layouts)
    cmask = np.zeros((BT * C, CO), dtype=F32)       # [(b,c'), (c,o)]
    for b in range(BT):
        for c in range(C):
            cmask[b * C + c, c * O:(c + 1) * O] = 1.0
    # maskz[p=(b,i), (b',c)] = 1 iff b' == b
    maskz = np.zeros((128, BT * C), dtype=F32)
    for b in range(BT):
        for c in range(C):
            maskz[b * ISUB:(b + 1) * ISUB, b * C + c] = 1.0
    sel = np.zeros((BT * C, BT), dtype=F32)         # [(b,c'), b2]
    for b in range(BT):
        for c in range(C):
            sel[b * C + c, b] = 1.0

    if USE_BF16:
        from ml_dtypes import bfloat16
        xz = xz.astype(bfloat16)
        xt = xt.astype(bfloat16)
        wr = wr.astype(bfloat16)
        maskz = maskz.astype(bfloat16)
    return xz, xt, wr, cmask, maskz, sel


def kernel(x: np.ndarray, W: np.ndarray) -> np.ndarray:
    from concourse import bass_utils

    if "nc" not in _compiled:
        _compiled["nc"] = _build_program()
    nc = _compiled["nc"]

    xz, xt, wr, cmask, maskz, sel = _prep_inputs(np.asarray(x), np.asarray(W))
    in_maps = [{"xz": xz[c], "xt": xt[c], "w": wr,
                "cmask": cmask, "maskz": maskz, "sel": sel}
               for c in range(NCORES)]
    res = bass_utils.run_bass_kernel_spmd(nc, in_maps, list(range(NCORES)))
    out = np.concatenate([res.results[c]["out"] for c in range(NCORES)], axis=0)
    return out.reshape(B, C, O)
